# revision 10
# baseline (speedup 1.0000x reference)
"""Distributed causal self-attention kernel for 8 Trainium2 NeuronCores.

Problem: B=4, T=2048, C=1024, H=16 heads, D=64 head dim, fp32.
  qkv = x @ W_qkv.T + b_qkv; causal attention per head; out = attn @ W_proj.T + b_proj

Sharding (hybrid DP x TP, no on-device collectives):
  core c -> batch b = c//2 (data parallel), head group g = c%2 (8 heads each,
  tensor parallel). Each core computes a row-parallel *partial* projection
  output for its batch; the host sums the two partials per batch (the TP
  reduction) and adds b_proj.

v2 vs baseline (353.7us):
  - all device tensors bf16 (x, W, outputs): halves DMA, same PE rate.
  - per-chunk tiles (x, qkT, attnT split at 512-column granularity) so the
    Tile scheduler's range deps never serialize whole phases.
  - emission order = scheduler priority: attention blocks are emitted as
    early as their data deps allow; v-pass remainder / next pair's qk /
    projection chunks are emitted AFTER so they fill PE slack instead of
    starving the ACT exp chain (the baseline's exp chain started at ~99us;
    target ~15us).
  - dummy EXP at t=0 pre-loads the ACT spline table (~2.7us) off-path.
  - wqkv columns regrouped pair-major ([q0|k0|q1|k1|...|v]) so pair 0's
    weights arrive in one early DMA.
  - b_proj folded on the host (it must sum TP partials anyway); outputs bf16.
  - one combined reciprocal-shift DMA per (pair, qc) instead of two.

  All matmuls bf16 (1 cyc/row): scores computed TRANSPOSED (no transposes
  anywhere); AV stationary [ones|V_h] emits softmax denominator + raw output
  in one matmul; 1/8 scale folded into W_q/b_q host-side.
"""
import sys

if "/opt/trn_rl_repo" not in sys.path:
    sys.path.insert(0, "/opt/trn_rl_repo")

import ml_dtypes
import numpy as np

import concourse.bass as bass
import concourse.tile as tile
from concourse import bacc, mybir
from concourse.bass_utils import run_bass_kernel_spmd
from concourse.masks import make_upper_triangular

F32 = mybir.dt.float32
BF16 = mybir.dt.bfloat16

B, T, C = 4, 2048, 1024
H, D = 16, 64
HC = 8            # heads per core
P = 128           # partitions
NCORES = 8
NT = T // P       # 16 t-tiles of 128
NTC = T // 512    # 4 t-chunks of 512
NCT = C // P      # 8 contraction tiles for qkv
JQK = 1024        # q+k columns per core
JV = 512          # v columns per core
NMT = C // P      # 8 proj output row tiles
NPJ = JV // P     # 4 proj contraction tiles

_compiled = None


def build():
    nc = bacc.Bacc("TRN2", target_bir_lowering=False, debug=False,
                   num_devices=NCORES)
    x_ext = nc.declare_dram_parameter("xT", [C, T], BF16, isOutput=False)
    # columns pair-major: [q0|k0|q1|k1|q2|k2|q3|k3 | v]
    wqkv_ext = nc.declare_dram_parameter("wqkv", [C, 3 * JV], BF16, isOutput=False)
    bqkv_ext = nc.declare_dram_parameter("bqkv", [3 * JV], F32, isOutput=False)
    wproj_ext = nc.declare_dram_parameter("wproj", [JV, C], BF16, isOutput=False)
    out_ext = nc.declare_dram_parameter("out", [C, T], BF16, isOutput=True)

    with tile.TileContext(nc, pool_alloc_mode="queue") as tc:
        _body(nc, tc, x_ext, wqkv_ext, bqkv_ext, wproj_ext, out_ext)
    nc.compile()
    return nc


def _body(nc, tc, x_ext, wqkv_ext, bqkv_ext, wproj_ext, out_ext):
    dma = nc.default_dma_engine

    from contextlib import ExitStack
    ctx = ExitStack()
    with ctx:
        singles = ctx.enter_context(tc.tile_pool(name="singles", bufs=1))
        qkt_pool = ctx.enter_context(tc.tile_pool(name="qkT", bufs=1))
        vpool = ctx.enter_context(tc.tile_pool(name="v", bufs=1))
        apool = ctx.enter_context(tc.tile_pool(name="attnT", bufs=1))
        ptpool = ctx.enter_context(tc.tile_pool(name="pt", bufs=6))
        rspool = ctx.enter_context(tc.tile_pool(name="rs", bufs=4))
        wp_pool = ctx.enter_context(tc.tile_pool(name="wp", bufs=1))
        opool = ctx.enter_context(tc.tile_pool(name="outs", bufs=4))
        psum = ctx.enter_context(tc.tile_pool(name="psum", bufs=1, space="PSUM"))
        xpool = ctx.enter_context(tc.tile_pool(name="x", bufs=1, side="right"))
        wv_pool = ctx.enter_context(tc.tile_pool(name="wv", bufs=1, side="right"))
        wqk_pool = ctx.enter_context(tc.tile_pool(name="wqk", bufs=1, side="right"))

        # ---- HAM warmup: dummy fp32 matmuls (4 cyc/row, ~1.7us each when
        # cold) bridge the initial DMA ramp so the PE clock is at 8/8 when
        # the first real matmuls issue. Plus a dummy EXP to pre-load the
        # ACT spline table set (~2.7us) while DMA streams in.
        warm = rspool.tile([P, 1024], F32, tag="rs", name="warm")
        nc.vector.memset(warm[:], 1.0)
        for i in range(6):
            wps = psum.tile([P, 512], F32, tag="mm", bufs=2, name=f"warm{i}")
            nc.tensor.matmul(wps[:], warm[:, 0:P], warm[:, 0:512])
        exp_dummy = singles.tile([P, 8], BF16)
        nc.scalar.activation(exp_dummy[:], warm[:, 0:8],
                             mybir.ActivationFunctionType.Exp)

        # ---- constants ----
        mask = singles.tile([P, P], BF16)       # upper-tri (t_q >= t_k) 0/1
        make_upper_triangular(nc, mask[:], val=1.0, diag=True)

        bqk_t = singles.tile([P, JQK // P], F32)  # per-partition q/k biases
        dma.dma_start(out=bqk_t[:], in_=bqkv_ext[:JQK].rearrange("(j p) -> p j", p=P))
        bv_b = singles.tile([P, JV], F32)       # v bias broadcast over partitions
        bv_src = bass.AP(tensor=bqkv_ext, offset=JQK, ap=[[0, P], [1, JV]])
        dma.dma_start(out=bv_b[:], in_=bv_src)

        # ---- input DMAs, in need-order (queue priority = emission order):
        # the critical 1.25MB (x tcn0 + pair-0 qk weights) streams first so
        # the first qk groups -- and with them the ACT exp chain -- start
        # as early as possible; wv/v-pass follows.
        # x: per (ct, tcn) tiles of [128, 512]
        xts = [[None] * NTC for _ in range(NCT)]

        def load_x(tcn):
            for ct in range(NCT):
                t_ = xpool.tile([P, 512], BF16, tag=f"x{ct}_{tcn}",
                                name=f"x{ct}_{tcn}")
                dma.dma_start(
                    out=t_[:],
                    in_=x_ext[ct * P:(ct + 1) * P, tcn * 512:(tcn + 1) * 512])
                xts[ct][tcn] = t_

        load_x(0)
        # wqk: [128, 1024] per ct, pair-major columns; pair0 slice first
        wqk = []
        for ct in range(NCT):
            wt = wqk_pool.tile([P, JQK], BF16, tag=f"wqk{ct}", name=f"wqk{ct}")
            dma.dma_start(out=wt[:, 0:256],
                          in_=wqkv_ext[ct * P:(ct + 1) * P, 0:256])
            wqk.append(wt)
        wv = []
        for ct in range(NCT):
            wt = wv_pool.tile([P, JV], BF16, tag=f"wv{ct}", name=f"wv{ct}")
            dma.dma_start(out=wt[:], in_=wqkv_ext[ct * P:(ct + 1) * P, JQK:])
            wv.append(wt)
        for tcn in range(1, NTC):
            load_x(tcn)
        for ct in range(NCT):
            dma.dma_start(out=wqk[ct][:, 256:],
                          in_=wqkv_ext[ct * P:(ct + 1) * P, 256:JQK])
        wproj = []
        for jt in range(NPJ):
            wt = wp_pool.tile([P, C], BF16, tag=f"wp{jt}", name=f"wp{jt}")
            dma.dma_start(out=wt[:], in_=wproj_ext[jt * P:(jt + 1) * P, :])
            wproj.append(wt)

        # ---- v pass ----
        # v_sb[kt]: [128, 1024] = per head h: [ones(64) | V_h(64)] at col 128h;
        # ones make attn@V emit the softmax denominator on partitions 0-63
        vts = []

        def v_tile(kt):
            psv = psum.tile([P, JV], F32, tag="mm", bufs=2, name=f"psv{kt}")
            for ct in range(NCT):
                nc.tensor.matmul(
                    psv[:],
                    xts[ct][kt // 4][:, (kt % 4) * P:(kt % 4 + 1) * P],
                    wv[ct][:],
                    start=(ct == 0), stop=(ct == NCT - 1),
                )
            vt = vpool.tile([P, 2 * JV], BF16, tag=f"v{kt}", name=f"v{kt}")
            vt3 = vt[:].rearrange("p (h c) -> p h c", h=HC)
            nc.gpsimd.memset(vt3[:, :, 0:64], 1.0)
            nc.vector.tensor_add(
                vt3[:, :, 64:128],
                psv[:].rearrange("p (h c) -> p h c", h=HC),
                bv_b[:].rearrange("p (h c) -> p h c", h=HC),
            )
            vts.append(vt)

        # ---- qk pass: per (pair, q/k, tcn) accumulation groups ----
        # qkT[p][0][tcn] = q^T chunk, qkT[p][1][tcn] = k^T chunk, [128, 512]
        qkT = {}

        def qk_group(p_, tcn):
            for w_ in range(2):   # 0=q, 1=k
                tname = ("q", "k")[w_]
                t_ = qkt_pool.tile([P, 512], BF16,
                                   tag=f"{tname}{p_ % 2}_{tcn}",
                                   name=f"{tname}T{p_}_{tcn}")
                ps = psum.tile([P, 512], F32, tag="mm", bufs=2,
                               name=f"psqk{p_}_{w_}_{tcn}")
                cols = 256 * p_ + 128 * w_
                for ct in range(NCT):
                    nc.tensor.matmul(
                        ps[:], wqk[ct][:, cols:cols + 128],
                        xts[ct][tcn][:],
                        start=(ct == 0), stop=(ct == NCT - 1),
                    )
                nc.vector.tensor_scalar_add(
                    out=t_[:], in0=ps[:],
                    scalar1=bqk_t[:, 2 * p_ + w_:2 * p_ + w_ + 1])
                qkT[p_][w_][tcn] = t_

        def qk_tiles(p_):
            qkT[p_] = [[None] * NTC, [None] * NTC]
            for tcn in range(NTC):
                qk_group(p_, tcn)

        # attnT[p][tcn]: [128 (2 heads x 64 d), 512] bf16
        attnT = {p_: [None] * NTC for p_ in range(4)}

        def attention(p_, qc):
            qT = qkT[p_][0][qc]
            pso = [psum.tile([P, 512], F32, tag="o", bufs=2,
                             name=f"pso{p_}_{qc}_{i}")
                   for i in range(2)]
            nkt = 4 * (qc + 1)
            # AV runs two blocks behind exp: by the time an AV reaches the
            # head of the PE FIFO its exp semaphore is already satisfied, so
            # the in-order engine queue never stalls on the exp edge
            pending = []
            for kt in range(nkt):
                o = max(0, kt * P - qc * 512)
                kT = qkT[p_][1][kt // 4]
                kcol = (kt % 4) * P
                ss = psum.tile([P, 1024], F32, tag="s", bufs=2,
                               name=f"pss{p_}_{qc}_{kt}")
                for h in range(2):
                    lo, hi = h * 64, (h + 1) * 64
                    nc.tensor.matmul(
                        ss[:, 512 * h + o:512 * (h + 1)],
                        kT[lo:hi, kcol:kcol + P],
                        qT[lo:hi, o:512],
                    )
                pt = ptpool.tile([P, 1024], BF16, tag="pt",
                                 name=f"pt{p_}_{qc}_{kt}")
                ss3 = ss[:].rearrange("p (h w) -> p h w", h=2)
                pt3 = pt[:].rearrange("p (h w) -> p h w", h=2)
                nc.scalar.activation(
                    pt3[:, :, o:], ss3[:, :, o:],
                    mybir.ActivationFunctionType.Exp,
                )
                if kt >= 4 * qc:
                    # diagonal block: zero t_q < t_k (both heads); on GpSimd
                    # (otherwise idle) to keep the DVE FIFO short
                    mask_b = bass.AP(
                        tensor=mask[:].tensor, offset=mask[:].offset,
                        ap=[mask[:].ap[0], [0, 2], [1, P]])
                    nc.gpsimd.tensor_mul(
                        pt3[:, :, o:o + P], pt3[:, :, o:o + P], mask_b)
                pending.append((pt, o, kt))
                if len(pending) > 2:
                    _emit_av(nc, vts, pso, p_, *pending.pop(0), nkt)
            for pend in pending:
                _emit_av(nc, vts, pso, p_, *pend, nkt)
            # normalize: pso rows 0:64 = row-sum, 64:128 = outT
            at = apool.tile([P, 512], BF16, tag=f"a{p_}_{qc}",
                            name=f"attnT{p_}_{qc}")
            attnT[p_][qc] = at
            rsb = rspool.tile([P, 1024], F32, tag="rs", name=f"rs{p_}_{qc}")
            # fast recip is lane-locked: compute at base 0 (frees the psum
            # fast), one combined DMA-shift to partitions 64-127
            nc.vector.reciprocal_approx_fast(rsb[0:64, 0:512], pso[0][0:64, :])
            nc.vector.reciprocal_approx_fast(rsb[0:64, 512:], pso[1][0:64, :])
            dma.dma_start(out=rsb[64:128, :], in_=rsb[0:64, :])
            for h in range(2):
                nc.vector.tensor_mul(
                    at[64 * h:64 * h + 64, :],
                    pso[h][64:128, :], rsb[64:128, 512 * h:512 * h + 512])

        def proj_chunk(tcn):
            for mt in range(NMT):
                psp = psum.tile([P, 512], F32, tag="mm", bufs=2,
                                name=f"psp{mt}_{tcn}")
                for jt in range(NPJ):
                    nc.tensor.matmul(
                        psp[:], wproj[jt][:, mt * P:(mt + 1) * P],
                        attnT[jt][tcn][:],
                        start=(jt == 0), stop=(jt == NPJ - 1),
                    )
                ot = opool.tile([P, 512], BF16, tag="ot", name=f"ot{mt}_{tcn}")
                nc.vector.tensor_scalar_add(out=ot[:], in0=psp[:], scalar1=0.0)
                dma.dma_start(
                    out=out_ext[mt * P:(mt + 1) * P,
                                tcn * 512:(tcn + 1) * 512],
                    in_=ot[:])

        # ---- main emission: attention as early as deps allow; fill work
        # (v remainder, next pair's qk, proj) emitted after = lower priority
        qkT[0] = [[None] * NTC, [None] * NTC]
        qk_group(0, 0)          # just tcn0: unblocks attention(0,0) asap
        for kt in range(4):
            v_tile(kt)
        attention(0, 0)
        for tcn in range(1, NTC):
            qk_group(0, tcn)
        for qc in range(1, NTC):
            # v tiles for this chunk, emitted just ahead of it: they fill PE
            # slack during the ACT-bound exp chain without outranking the
            # already-emitted attention pipeline in scheduler priority
            for kt in range(4 * qc, 4 * qc + 4):
                v_tile(kt)
            attention(0, qc)
        for p_ in (1, 2):
            qk_tiles(p_)
            for qc in range(NTC):
                attention(p_, qc)
        qk_tiles(3)
        # pair 3 runs q-chunks high-to-low; each chunk completes attnT[*][qc]
        # so its projection chunk is emitted (and runs) immediately after,
        # leaving only proj(qc=0) as the tail
        for qc in range(NTC - 1, -1, -1):
            attention(3, qc)
            proj_chunk(qc)


def _emit_av(nc, vts, pso, p_, pt, o, kt, nkt):
    """attn@V for one (pair, kt) block: [ones|V_h].T @ P~ accumulated."""
    for h in range(2):
        head = 2 * p_ + h
        vaug = vts[kt][:, 128 * head:128 * head + 128]
        nc.tensor.matmul(
            pso[h][:, o:], vaug, pt[:, 512 * h + o:512 * (h + 1)],
            start=(kt == 0), stop=(kt == nkt - 1),
        )


def shard_inputs(x, W_qkv, b_qkv, W_proj, b_proj):
    """Build the 8 per-core input maps (host-side sharding)."""
    x = np.asarray(x, np.float32)
    W_qkv = np.asarray(W_qkv, np.float32)
    b_qkv = np.asarray(b_qkv, np.float32)
    W_proj = np.asarray(W_proj, np.float32)
    in_maps = []
    for c in range(NCORES):
        b, g = c // 2, c % 2
        s = slice(512 * g, 512 * g + 512)
        Wq = W_qkv[0 * C:1 * C][s] * 0.125
        Wk = W_qkv[1 * C:2 * C][s]
        Wv = W_qkv[2 * C:3 * C][s]
        bq = b_qkv[0 * C:1 * C][s] * 0.125
        bk = b_qkv[1 * C:2 * C][s]
        # pair-major interleave: [q0|k0|q1|k1|q2|k2|q3|k3|v]
        qk_rows = []
        bqk = []
        for p_ in range(4):
            qk_rows.append(Wq[128 * p_:128 * (p_ + 1)])
            qk_rows.append(Wk[128 * p_:128 * (p_ + 1)])
            bqk.append(bq[128 * p_:128 * (p_ + 1)])
            bqk.append(bk[128 * p_:128 * (p_ + 1)])
        wqkv = np.ascontiguousarray(
            np.concatenate(qk_rows + [Wv], 0).T).astype(ml_dtypes.bfloat16)
        bv = b_qkv[2 * C:3 * C][s]
        in_maps.append({
            "xT": np.ascontiguousarray(x[b].T).astype(ml_dtypes.bfloat16),
            "wqkv": wqkv,
            "bqkv": np.ascontiguousarray(np.concatenate(bqk + [bv])),
            "wproj": np.ascontiguousarray(W_proj[:, s].T).astype(ml_dtypes.bfloat16),
        })
    return in_maps


def run(in_maps, trace=False):
    global _compiled
    if _compiled is None:
        _compiled = build()
    return run_bass_kernel_spmd(
        _compiled, in_maps, core_ids=list(range(NCORES)), trace=trace)


def postprocess(res, b_proj):
    b_proj = np.asarray(b_proj, np.float32)
    out = np.empty((B, T, C), np.float32)
    for b in range(B):
        partial = (res.results[2 * b]["out"].astype(np.float32)
                   + res.results[2 * b + 1]["out"].astype(np.float32))
        out[b] = partial.T + b_proj
    return out


def kernel(x, W_qkv, b_qkv, W_proj, b_proj):
    in_maps = shard_inputs(x, W_qkv, b_qkv, W_proj, b_proj)
    res = run(in_maps)
    return postprocess(res, b_proj)


if __name__ == "__main__":
    rng = np.random.default_rng(0)
    xs = {
        "x": rng.standard_normal((B, T, C)).astype(np.float32),
        "W_qkv": (rng.standard_normal((3 * C, C)) / 32).astype(np.float32),
        "b_qkv": (rng.standard_normal(3 * C) * 0.02).astype(np.float32),
        "W_proj": (rng.standard_normal((C, C)) / 32).astype(np.float32),
        "b_proj": (rng.standard_normal(C) * 0.02).astype(np.float32),
    }
    out = kernel(**xs)
    print("out", out.shape, out.dtype, np.abs(out).mean())


# revision 11
# speedup vs baseline: 1.0317x; 1.0317x over previous
"""Distributed causal self-attention kernel for 8 Trainium2 NeuronCores.

Problem: B=4, T=2048, C=1024, H=16 heads, D=64 head dim, fp32.
  qkv = x @ W_qkv.T + b_qkv; causal attention per head; out = attn @ W_proj.T + b_proj

Sharding (hybrid DP x TP, no on-device collectives):
  core c -> batch b = c//2 (data parallel), head group g = c%2 (8 heads each,
  tensor parallel). Each core computes a row-parallel *partial* projection
  output for its batch; the host sums the two partials per batch (the TP
  reduction) and adds b_proj.

v2 vs baseline (353.7us):
  - all device tensors bf16 (x, W, outputs): halves DMA, same PE rate.
  - per-chunk tiles (x, qkT, attnT split at 512-column granularity) so the
    Tile scheduler's range deps never serialize whole phases.
  - emission order = scheduler priority: attention blocks are emitted as
    early as their data deps allow; v-pass remainder / next pair's qk /
    projection chunks are emitted AFTER so they fill PE slack instead of
    starving the ACT exp chain (the baseline's exp chain started at ~99us;
    target ~15us).
  - dummy EXP at t=0 pre-loads the ACT spline table (~2.7us) off-path.
  - wqkv columns regrouped pair-major ([q0|k0|q1|k1|...|v]) so pair 0's
    weights arrive in one early DMA.
  - b_proj folded on the host (it must sum TP partials anyway); outputs bf16.
  - one combined reciprocal-shift DMA per (pair, qc) instead of two.

  All matmuls bf16 (1 cyc/row): scores computed TRANSPOSED (no transposes
  anywhere); AV stationary [ones|V_h] emits softmax denominator + raw output
  in one matmul; 1/8 scale folded into W_q/b_q host-side.
"""
import sys

if "/opt/trn_rl_repo" not in sys.path:
    sys.path.insert(0, "/opt/trn_rl_repo")

import ml_dtypes
import numpy as np

import concourse.bass as bass
import concourse.tile as tile
from concourse import bacc, mybir
from concourse.bass_utils import run_bass_kernel_spmd
from concourse.masks import make_upper_triangular

F32 = mybir.dt.float32
BF16 = mybir.dt.bfloat16

B, T, C = 4, 2048, 1024
H, D = 16, 64
HC = 8            # heads per core
P = 128           # partitions
NCORES = 8
NT = T // P       # 16 t-tiles of 128
NTC = T // 512    # 4 t-chunks of 512
NCT = C // P      # 8 contraction tiles for qkv
JQK = 1024        # q+k columns per core
JV = 512          # v columns per core
NMT = C // P      # 8 proj output row tiles
NPJ = JV // P     # 4 proj contraction tiles

_compiled = None


def build():
    nc = bacc.Bacc("TRN2", target_bir_lowering=False, debug=False,
                   num_devices=NCORES)
    x_ext = nc.declare_dram_parameter("xT", [C, T], BF16, isOutput=False)
    # columns pair-major: [q0|k0|q1|k1|q2|k2|q3|k3 | v]
    wqkv_ext = nc.declare_dram_parameter("wqkv", [C, 3 * JV], BF16, isOutput=False)
    bqkv_ext = nc.declare_dram_parameter("bqkv", [3 * JV], F32, isOutput=False)
    wproj_ext = nc.declare_dram_parameter("wproj", [JV, C], BF16, isOutput=False)
    out_ext = nc.declare_dram_parameter("out", [C, T], BF16, isOutput=True)

    with tile.TileContext(nc, pool_alloc_mode="queue") as tc:
        _body(nc, tc, x_ext, wqkv_ext, bqkv_ext, wproj_ext, out_ext)
    nc.compile()
    return nc


def _body(nc, tc, x_ext, wqkv_ext, bqkv_ext, wproj_ext, out_ext):
    dma = nc.default_dma_engine

    from contextlib import ExitStack
    ctx = ExitStack()
    with ctx:
        singles = ctx.enter_context(tc.tile_pool(name="singles", bufs=1))
        qkt_pool = ctx.enter_context(tc.tile_pool(name="qkT", bufs=1))
        vpool = ctx.enter_context(tc.tile_pool(name="v", bufs=1))
        apool = ctx.enter_context(tc.tile_pool(name="attnT", bufs=1))
        ptpool = ctx.enter_context(tc.tile_pool(name="pt", bufs=6))
        rspool = ctx.enter_context(tc.tile_pool(name="rs", bufs=4))
        wp_pool = ctx.enter_context(tc.tile_pool(name="wp", bufs=1))
        opool = ctx.enter_context(tc.tile_pool(name="outs", bufs=4))
        psum = ctx.enter_context(tc.tile_pool(name="psum", bufs=1, space="PSUM"))
        xpool = ctx.enter_context(tc.tile_pool(name="x", bufs=1, side="right"))
        wv_pool = ctx.enter_context(tc.tile_pool(name="wv", bufs=1, side="right"))
        wqk_pool = ctx.enter_context(tc.tile_pool(name="wqk", bufs=1, side="right"))

        # ---- HAM warmup: dummy fp32 matmuls (4 cyc/row, ~1.7us each when
        # cold) bridge the initial DMA ramp so the PE clock is at 8/8 when
        # the first real matmuls issue. Plus a dummy EXP to pre-load the
        # ACT spline table set (~2.7us) while DMA streams in.
        warm = rspool.tile([P, 1024], F32, tag="rs", name="warm")
        nc.vector.memset(warm[:], 1.0)
        for i in range(6):
            wps = psum.tile([P, 512], F32, tag="mm", bufs=2, name=f"warm{i}")
            nc.tensor.matmul(wps[:], warm[:, 0:P], warm[:, 0:512])
        exp_dummy = singles.tile([P, 8], BF16)
        nc.scalar.activation(exp_dummy[:], warm[:, 0:8],
                             mybir.ActivationFunctionType.Exp)

        # ---- constants ----
        mask = singles.tile([P, P], BF16)       # upper-tri (t_q >= t_k) 0/1
        make_upper_triangular(nc, mask[:], val=1.0, diag=True)

        bqk_t = singles.tile([P, JQK // P], F32)  # per-partition q/k biases
        dma.dma_start(out=bqk_t[:], in_=bqkv_ext[:JQK].rearrange("(j p) -> p j", p=P))
        bv_b = singles.tile([P, JV], F32)       # v bias broadcast over partitions
        bv_src = bass.AP(tensor=bqkv_ext, offset=JQK, ap=[[0, P], [1, JV]])
        dma.dma_start(out=bv_b[:], in_=bv_src)

        # ---- input DMAs, in need-order (queue priority = emission order):
        # the critical 1.25MB (x tcn0 + pair-0 qk weights) streams first so
        # the first qk groups -- and with them the ACT exp chain -- start
        # as early as possible; wv/v-pass follows.
        # x: per (ct, tcn) tiles of [128, 512]
        xts = [[None] * NTC for _ in range(NCT)]

        def load_x(tcn):
            for ct in range(NCT):
                t_ = xpool.tile([P, 512], BF16, tag=f"x{ct}_{tcn}",
                                name=f"x{ct}_{tcn}")
                dma.dma_start(
                    out=t_[:],
                    in_=x_ext[ct * P:(ct + 1) * P, tcn * 512:(tcn + 1) * 512])
                xts[ct][tcn] = t_

        load_x(0)
        # wqk: [128, 1024] per ct, pair-major columns; pair0 slice first
        wqk = []
        for ct in range(NCT):
            wt = wqk_pool.tile([P, JQK], BF16, tag=f"wqk{ct}", name=f"wqk{ct}")
            dma.dma_start(out=wt[:, 0:256],
                          in_=wqkv_ext[ct * P:(ct + 1) * P, 0:256])
            wqk.append(wt)
        wv = []
        for ct in range(NCT):
            wt = wv_pool.tile([P, JV], BF16, tag=f"wv{ct}", name=f"wv{ct}")
            dma.dma_start(out=wt[:], in_=wqkv_ext[ct * P:(ct + 1) * P, JQK:])
            wv.append(wt)
        for tcn in range(1, NTC):
            load_x(tcn)
        for ct in range(NCT):
            dma.dma_start(out=wqk[ct][:, 256:],
                          in_=wqkv_ext[ct * P:(ct + 1) * P, 256:JQK])
        wproj = []
        for jt in range(NPJ):
            wt = wp_pool.tile([P, C], BF16, tag=f"wp{jt}", name=f"wp{jt}")
            dma.dma_start(out=wt[:], in_=wproj_ext[jt * P:(jt + 1) * P, :])
            wproj.append(wt)

        # ---- v pass ----
        # v_sb[kt]: [128, 1024] = per head h: [ones(64) | V_h(64)] at col 128h;
        # ones make attn@V emit the softmax denominator on partitions 0-63
        vts = []

        def v_tile(kt):
            psv = psum.tile([P, JV], F32, tag="mm", bufs=2, name=f"psv{kt}")
            for ct in range(NCT):
                nc.tensor.matmul(
                    psv[:],
                    xts[ct][kt // 4][:, (kt % 4) * P:(kt % 4 + 1) * P],
                    wv[ct][:],
                    start=(ct == 0), stop=(ct == NCT - 1),
                )
            vt = vpool.tile([P, 2 * JV], BF16, tag=f"v{kt}", name=f"v{kt}")
            vt3 = vt[:].rearrange("p (h c) -> p h c", h=HC)
            nc.gpsimd.memset(vt3[:, :, 0:64], 1.0)
            nc.vector.tensor_add(
                vt3[:, :, 64:128],
                psv[:].rearrange("p (h c) -> p h c", h=HC),
                bv_b[:].rearrange("p (h c) -> p h c", h=HC),
            )
            vts.append(vt)

        # ---- qk pass: per (pair, q/k, tcn) accumulation groups ----
        # qkT[p][0][tcn] = q^T chunk, qkT[p][1][tcn] = k^T chunk, [128, 512]
        qkT = {}

        def qk_group(p_, tcn):
            for w_ in range(2):   # 0=q, 1=k
                tname = ("q", "k")[w_]
                t_ = qkt_pool.tile([P, 512], BF16,
                                   tag=f"{tname}{p_ % 2}_{tcn}",
                                   name=f"{tname}T{p_}_{tcn}")
                ps = psum.tile([P, 512], F32, tag="mm", bufs=2,
                               name=f"psqk{p_}_{w_}_{tcn}")
                cols = 256 * p_ + 128 * w_
                for ct in range(NCT):
                    nc.tensor.matmul(
                        ps[:], wqk[ct][:, cols:cols + 128],
                        xts[ct][tcn][:],
                        start=(ct == 0), stop=(ct == NCT - 1),
                    )
                nc.vector.tensor_scalar_add(
                    out=t_[:], in0=ps[:],
                    scalar1=bqk_t[:, 2 * p_ + w_:2 * p_ + w_ + 1])
                qkT[p_][w_][tcn] = t_

        def qk_tiles(p_):
            qkT[p_] = [[None] * NTC, [None] * NTC]
            for tcn in range(NTC):
                qk_group(p_, tcn)

        # attnT[p][tcn]: [128 (2 heads x 64 d), 512] bf16
        attnT = {p_: [None] * NTC for p_ in range(4)}

        def attention(p_, qc):
            qT = qkT[p_][0][qc]
            pso = [psum.tile([P, 512], F32, tag="o", bufs=2,
                             name=f"pso{p_}_{qc}_{i}")
                   for i in range(2)]
            nkt = 4 * (qc + 1)
            # AV runs two blocks behind exp: by the time an AV reaches the
            # head of the PE FIFO its exp semaphore is already satisfied, so
            # the in-order engine queue never stalls on the exp edge
            pending = []
            for kt in range(nkt):
                o = max(0, kt * P - qc * 512)
                kT = qkT[p_][1][kt // 4]
                kcol = (kt % 4) * P
                ss = psum.tile([P, 1024], F32, tag="s", bufs=2,
                               name=f"pss{p_}_{qc}_{kt}")
                for h in range(2):
                    lo, hi = h * 64, (h + 1) * 64
                    nc.tensor.matmul(
                        ss[:, 512 * h + o:512 * (h + 1)],
                        kT[lo:hi, kcol:kcol + P],
                        qT[lo:hi, o:512],
                    )
                pt = ptpool.tile([P, 1024], BF16, tag="pt",
                                 name=f"pt{p_}_{qc}_{kt}")
                ss3 = ss[:].rearrange("p (h w) -> p h w", h=2)
                pt3 = pt[:].rearrange("p (h w) -> p h w", h=2)
                nc.scalar.activation(
                    pt3[:, :, o:], ss3[:, :, o:],
                    mybir.ActivationFunctionType.Exp,
                )
                if kt >= 4 * qc:
                    # diagonal block: zero t_q < t_k (both heads)
                    mask_b = bass.AP(
                        tensor=mask[:].tensor, offset=mask[:].offset,
                        ap=[mask[:].ap[0], [0, 2], [1, P]])
                    nc.vector.tensor_mul(
                        pt3[:, :, o:o + P], pt3[:, :, o:o + P], mask_b)
                pending.append((pt, o, kt))
                if len(pending) > 2:
                    _emit_av(nc, vts, pso, p_, *pending.pop(0), nkt)
            for pend in pending:
                _emit_av(nc, vts, pso, p_, *pend, nkt)
            # normalize: pso rows 0:64 = row-sum, 64:128 = outT
            at = apool.tile([P, 512], BF16, tag=f"a{p_}_{qc}",
                            name=f"attnT{p_}_{qc}")
            attnT[p_][qc] = at
            rsb = rspool.tile([P, 1024], F32, tag="rs", name=f"rs{p_}_{qc}")
            # fast recip is lane-locked: compute at base 0 (frees the psum
            # fast), one combined DMA-shift to partitions 64-127
            nc.vector.reciprocal_approx_fast(rsb[0:64, 0:512], pso[0][0:64, :])
            nc.vector.reciprocal_approx_fast(rsb[0:64, 512:], pso[1][0:64, :])
            dma.dma_start(out=rsb[64:128, :], in_=rsb[0:64, :])
            for h in range(2):
                nc.vector.tensor_mul(
                    at[64 * h:64 * h + 64, :],
                    pso[h][64:128, :], rsb[64:128, 512 * h:512 * h + 512])

        def proj_chunk(tcn):
            for mt in range(NMT):
                psp = psum.tile([P, 512], F32, tag="mm", bufs=2,
                                name=f"psp{mt}_{tcn}")
                for jt in range(NPJ):
                    nc.tensor.matmul(
                        psp[:], wproj[jt][:, mt * P:(mt + 1) * P],
                        attnT[jt][tcn][:],
                        start=(jt == 0), stop=(jt == NPJ - 1),
                    )
                ot = opool.tile([P, 512], BF16, tag="ot", name=f"ot{mt}_{tcn}")
                nc.vector.tensor_scalar_add(out=ot[:], in0=psp[:], scalar1=0.0)
                dma.dma_start(
                    out=out_ext[mt * P:(mt + 1) * P,
                                tcn * 512:(tcn + 1) * 512],
                    in_=ot[:])

        # ---- main emission: attention as early as deps allow; fill work
        # (v remainder, next pair's qk, proj) emitted after = lower priority
        qkT[0] = [[None] * NTC, [None] * NTC]
        qk_group(0, 0)          # just tcn0: unblocks attention(0,0) asap
        for kt in range(4):
            v_tile(kt)
        attention(0, 0)
        for tcn in range(1, NTC):
            qk_group(0, tcn)
        for qc in range(1, NTC):
            # v tiles for this chunk, emitted just ahead of it: they fill PE
            # slack during the ACT-bound exp chain without outranking the
            # already-emitted attention pipeline in scheduler priority
            for kt in range(4 * qc, 4 * qc + 4):
                v_tile(kt)
            attention(0, qc)
        for p_ in (1, 2):
            qk_tiles(p_)
            for qc in range(NTC):
                attention(p_, qc)
        qk_tiles(3)
        # pair 3 runs q-chunks high-to-low; each chunk completes attnT[*][qc]
        # so its projection chunk is emitted (and runs) immediately after,
        # leaving only proj(qc=0) as the tail
        for qc in range(NTC - 1, -1, -1):
            attention(3, qc)
            proj_chunk(qc)


def _emit_av(nc, vts, pso, p_, pt, o, kt, nkt):
    """attn@V for one (pair, kt) block: [ones|V_h].T @ P~ accumulated."""
    for h in range(2):
        head = 2 * p_ + h
        vaug = vts[kt][:, 128 * head:128 * head + 128]
        nc.tensor.matmul(
            pso[h][:, o:], vaug, pt[:, 512 * h + o:512 * (h + 1)],
            start=(kt == 0), stop=(kt == nkt - 1),
        )


def shard_inputs(x, W_qkv, b_qkv, W_proj, b_proj):
    """Build the 8 per-core input maps (host-side sharding)."""
    x = np.asarray(x, np.float32)
    W_qkv = np.asarray(W_qkv, np.float32)
    b_qkv = np.asarray(b_qkv, np.float32)
    W_proj = np.asarray(W_proj, np.float32)
    in_maps = []
    for c in range(NCORES):
        b, g = c // 2, c % 2
        s = slice(512 * g, 512 * g + 512)
        Wq = W_qkv[0 * C:1 * C][s] * 0.125
        Wk = W_qkv[1 * C:2 * C][s]
        Wv = W_qkv[2 * C:3 * C][s]
        bq = b_qkv[0 * C:1 * C][s] * 0.125
        bk = b_qkv[1 * C:2 * C][s]
        # pair-major interleave: [q0|k0|q1|k1|q2|k2|q3|k3|v]
        qk_rows = []
        bqk = []
        for p_ in range(4):
            qk_rows.append(Wq[128 * p_:128 * (p_ + 1)])
            qk_rows.append(Wk[128 * p_:128 * (p_ + 1)])
            bqk.append(bq[128 * p_:128 * (p_ + 1)])
            bqk.append(bk[128 * p_:128 * (p_ + 1)])
        wqkv = np.ascontiguousarray(
            np.concatenate(qk_rows + [Wv], 0).T).astype(ml_dtypes.bfloat16)
        bv = b_qkv[2 * C:3 * C][s]
        in_maps.append({
            "xT": np.ascontiguousarray(x[b].T).astype(ml_dtypes.bfloat16),
            "wqkv": wqkv,
            "bqkv": np.ascontiguousarray(np.concatenate(bqk + [bv])),
            "wproj": np.ascontiguousarray(W_proj[:, s].T).astype(ml_dtypes.bfloat16),
        })
    return in_maps


def run(in_maps, trace=False):
    global _compiled
    if _compiled is None:
        _compiled = build()
    return run_bass_kernel_spmd(
        _compiled, in_maps, core_ids=list(range(NCORES)), trace=trace)


def postprocess(res, b_proj):
    b_proj = np.asarray(b_proj, np.float32)
    out = np.empty((B, T, C), np.float32)
    for b in range(B):
        partial = (res.results[2 * b]["out"].astype(np.float32)
                   + res.results[2 * b + 1]["out"].astype(np.float32))
        out[b] = partial.T + b_proj
    return out


def kernel(x, W_qkv, b_qkv, W_proj, b_proj):
    in_maps = shard_inputs(x, W_qkv, b_qkv, W_proj, b_proj)
    res = run(in_maps)
    return postprocess(res, b_proj)


if __name__ == "__main__":
    rng = np.random.default_rng(0)
    xs = {
        "x": rng.standard_normal((B, T, C)).astype(np.float32),
        "W_qkv": (rng.standard_normal((3 * C, C)) / 32).astype(np.float32),
        "b_qkv": (rng.standard_normal(3 * C) * 0.02).astype(np.float32),
        "W_proj": (rng.standard_normal((C, C)) / 32).astype(np.float32),
        "b_proj": (rng.standard_normal(C) * 0.02).astype(np.float32),
    }
    out = kernel(**xs)
    print("out", out.shape, out.dtype, np.abs(out).mean())


# revision 12
# speedup vs baseline: 1.1079x; 1.0739x over previous
"""Distributed causal self-attention kernel for 8 Trainium2 NeuronCores.

Problem: B=4, T=2048, C=1024, H=16 heads, D=64 head dim, fp32.
  qkv = x @ W_qkv.T + b_qkv; causal attention per head; out = attn @ W_proj.T + b_proj

Sharding (hybrid DP x TP, no on-device collectives):
  core c -> batch b = c//2 (data parallel), head group g = c%2 (8 heads each,
  tensor parallel). Each core computes a row-parallel *partial* projection
  output for its batch; the host sums the two partials per batch (the TP
  reduction) and adds b_proj.

v2 vs baseline (353.7us):
  - all device tensors bf16 (x, W, outputs): halves DMA, same PE rate.
  - per-chunk tiles (x, qkT, attnT split at 512-column granularity) so the
    Tile scheduler's range deps never serialize whole phases.
  - emission order = scheduler priority: attention blocks are emitted as
    early as their data deps allow; v-pass remainder / next pair's qk /
    projection chunks are emitted AFTER so they fill PE slack instead of
    starving the ACT exp chain (the baseline's exp chain started at ~99us;
    target ~15us).
  - dummy EXP at t=0 pre-loads the ACT spline table (~2.7us) off-path.
  - wqkv columns regrouped pair-major ([q0|k0|q1|k1|...|v]) so pair 0's
    weights arrive in one early DMA.
  - b_proj folded on the host (it must sum TP partials anyway); outputs bf16.
  - one combined reciprocal-shift DMA per (pair, qc) instead of two.

  All matmuls bf16 (1 cyc/row): scores computed TRANSPOSED (no transposes
  anywhere); AV stationary [ones|V_h] emits softmax denominator + raw output
  in one matmul; 1/8 scale folded into W_q/b_q host-side.
"""
import sys

if "/opt/trn_rl_repo" not in sys.path:
    sys.path.insert(0, "/opt/trn_rl_repo")

import ml_dtypes
import numpy as np

import concourse.bass as bass
import concourse.tile as tile
from concourse import bacc, mybir
from concourse.bass_utils import run_bass_kernel_spmd
from concourse.masks import make_upper_triangular

F32 = mybir.dt.float32
BF16 = mybir.dt.bfloat16

B, T, C = 4, 2048, 1024
H, D = 16, 64
HC = 8            # heads per core
P = 128           # partitions
NCORES = 8
NT = T // P       # 16 t-tiles of 128
NTC = T // 512    # 4 t-chunks of 512
NCT = C // P      # 8 contraction tiles for qkv
JQK = 1024        # q+k columns per core
JV = 512          # v columns per core
NMT = C // P      # 8 proj output row tiles
NPJ = JV // P     # 4 proj contraction tiles

_compiled = None


def build():
    nc = bacc.Bacc("TRN2", target_bir_lowering=False, debug=False,
                   num_devices=NCORES)
    x_ext = nc.declare_dram_parameter("xT", [C, T], BF16, isOutput=False)
    # columns pair-major: [q0|k0|q1|k1|q2|k2|q3|k3 | v]
    wqkv_ext = nc.declare_dram_parameter("wqkv", [C, 3 * JV], BF16, isOutput=False)
    bqkv_ext = nc.declare_dram_parameter("bqkv", [3 * JV], F32, isOutput=False)
    wproj_ext = nc.declare_dram_parameter("wproj", [JV, C], BF16, isOutput=False)
    out_ext = nc.declare_dram_parameter("out", [C, T], BF16, isOutput=True)

    with tile.TileContext(nc, pool_alloc_mode="queue") as tc:
        _body(nc, tc, x_ext, wqkv_ext, bqkv_ext, wproj_ext, out_ext)
    nc.compile()
    return nc


def _body(nc, tc, x_ext, wqkv_ext, bqkv_ext, wproj_ext, out_ext):
    dma = nc.default_dma_engine

    from contextlib import ExitStack
    ctx = ExitStack()
    with ctx:
        singles = ctx.enter_context(tc.tile_pool(name="singles", bufs=1))
        qkt_pool = ctx.enter_context(tc.tile_pool(name="qkT", bufs=1))
        vpool = ctx.enter_context(tc.tile_pool(name="v", bufs=1))
        apool = ctx.enter_context(tc.tile_pool(name="attnT", bufs=1))
        ptpool = ctx.enter_context(tc.tile_pool(name="pt", bufs=6))
        rspool = ctx.enter_context(tc.tile_pool(name="rs", bufs=4))
        wp_pool = ctx.enter_context(tc.tile_pool(name="wp", bufs=1))
        opool = ctx.enter_context(tc.tile_pool(name="outs", bufs=4))
        psum = ctx.enter_context(tc.tile_pool(name="psum", bufs=1, space="PSUM"))
        xpool = ctx.enter_context(tc.tile_pool(name="x", bufs=1, side="right"))
        wv_pool = ctx.enter_context(tc.tile_pool(name="wv", bufs=1, side="right"))
        wqk_pool = ctx.enter_context(tc.tile_pool(name="wqk", bufs=1, side="right"))

        # ---- HAM warmup: dummy fp32 matmuls (4 cyc/row, ~1.7us each when
        # cold) bridge the initial DMA ramp so the PE clock is at 8/8 when
        # the first real matmuls issue. Plus a dummy EXP to pre-load the
        # ACT spline table set (~2.7us) while DMA streams in.
        warm = rspool.tile([P, 1024], F32, tag="rs", name="warm")
        nc.vector.memset(warm[:], 1.0)
        for i in range(6):
            wps = psum.tile([P, 512], F32, tag="mm", bufs=2, name=f"warm{i}")
            nc.tensor.matmul(wps[:], warm[:, 0:P], warm[:, 0:512])
        exp_dummy = singles.tile([P, 8], BF16)
        nc.scalar.activation(exp_dummy[:], warm[:, 0:8],
                             mybir.ActivationFunctionType.Exp)

        # ---- constants ----
        mask = singles.tile([P, P], BF16)       # upper-tri (t_q >= t_k) 0/1
        make_upper_triangular(nc, mask[:], val=1.0, diag=True)

        bqk_t = singles.tile([P, JQK // P], F32)  # per-partition q/k biases
        dma.dma_start(out=bqk_t[:], in_=bqkv_ext[:JQK].rearrange("(j p) -> p j", p=P))
        bv_b = singles.tile([P, JV], F32)       # v bias broadcast over partitions
        bv_src = bass.AP(tensor=bqkv_ext, offset=JQK, ap=[[0, P], [1, JV]])
        dma.dma_start(out=bv_b[:], in_=bv_src)

        # ---- input DMAs, in need-order (queue priority = emission order):
        # the critical 1.25MB (x tcn0 + pair-0 qk weights) streams first so
        # the first qk groups -- and with them the ACT exp chain -- start
        # as early as possible; wv/v-pass follows.
        # x: per (ct, tcn) tiles of [128, 512]
        xts = [[None] * NTC for _ in range(NCT)]

        def load_x(tcn):
            for ct in range(NCT):
                t_ = xpool.tile([P, 512], BF16, tag=f"x{ct}_{tcn}",
                                name=f"x{ct}_{tcn}")
                dma.dma_start(
                    out=t_[:],
                    in_=x_ext[ct * P:(ct + 1) * P, tcn * 512:(tcn + 1) * 512])
                xts[ct][tcn] = t_

        load_x(0)
        # wqk: [128, 1024] per ct, pair-major columns; pair0 slice first
        wqk = []
        for ct in range(NCT):
            wt = wqk_pool.tile([P, JQK], BF16, tag=f"wqk{ct}", name=f"wqk{ct}")
            dma.dma_start(out=wt[:, 0:256],
                          in_=wqkv_ext[ct * P:(ct + 1) * P, 0:256])
            wqk.append(wt)
        wv = []
        for ct in range(NCT):
            wt = wv_pool.tile([P, JV], BF16, tag=f"wv{ct}", name=f"wv{ct}")
            dma.dma_start(out=wt[:], in_=wqkv_ext[ct * P:(ct + 1) * P, JQK:])
            wv.append(wt)
        for tcn in range(1, NTC):
            load_x(tcn)
        for ct in range(NCT):
            dma.dma_start(out=wqk[ct][:, 256:],
                          in_=wqkv_ext[ct * P:(ct + 1) * P, 256:JQK])
        wproj = []
        for jt in range(NPJ):
            wt = wp_pool.tile([P, C], BF16, tag=f"wp{jt}", name=f"wp{jt}")
            dma.dma_start(out=wt[:], in_=wproj_ext[jt * P:(jt + 1) * P, :])
            wproj.append(wt)

        # ---- v pass ----
        # v_sb[kt]: [128, 1024] = per head h: [ones(64) | V_h(64)] at col 128h;
        # ones make attn@V emit the softmax denominator on partitions 0-63
        vts = []

        def v_tile(kt):
            psv = psum.tile([P, JV], F32, tag="mm", bufs=2, name=f"psv{kt}")
            for ct in range(NCT):
                nc.tensor.matmul(
                    psv[:],
                    xts[ct][kt // 4][:, (kt % 4) * P:(kt % 4 + 1) * P],
                    wv[ct][:],
                    start=(ct == 0), stop=(ct == NCT - 1),
                )
            vt = vpool.tile([P, 2 * JV], BF16, tag=f"v{kt}", name=f"v{kt}")
            vt3 = vt[:].rearrange("p (h c) -> p h c", h=HC)
            nc.gpsimd.memset(vt3[:, :, 0:64], 1.0)
            nc.vector.tensor_add(
                vt3[:, :, 64:128],
                psv[:].rearrange("p (h c) -> p h c", h=HC),
                bv_b[:].rearrange("p (h c) -> p h c", h=HC),
            )
            vts.append(vt)

        # ---- qk pass: per (pair, q/k, tcn) accumulation groups ----
        # qkT[p][0][tcn] = q^T chunk, qkT[p][1][tcn] = k^T chunk, [128, 512]
        qkT = {}

        def qk_group(p_, tcn):
            for w_ in range(2):   # 0=q, 1=k
                tname = ("q", "k")[w_]
                t_ = qkt_pool.tile([P, 512], BF16,
                                   tag=f"{tname}{p_ % 2}_{tcn}",
                                   name=f"{tname}T{p_}_{tcn}")
                ps = psum.tile([P, 512], F32, tag="mm", bufs=2,
                               name=f"psqk{p_}_{w_}_{tcn}")
                cols = 256 * p_ + 128 * w_
                for ct in range(NCT):
                    nc.tensor.matmul(
                        ps[:], wqk[ct][:, cols:cols + 128],
                        xts[ct][tcn][:],
                        start=(ct == 0), stop=(ct == NCT - 1),
                    )
                nc.vector.tensor_scalar_add(
                    out=t_[:], in0=ps[:],
                    scalar1=bqk_t[:, 2 * p_ + w_:2 * p_ + w_ + 1])
                qkT[p_][w_][tcn] = t_

        def qk_tiles(p_):
            qkT[p_] = [[None] * NTC, [None] * NTC]
            for tcn in range(NTC):
                qk_group(p_, tcn)

        # attnT[p][tcn]: [128 (2 heads x 64 d), 512] bf16
        attnT = {p_: [None] * NTC for p_ in range(4)}

        def attention(p_, qc):
            qT = qkT[p_][0][qc]
            pso = [psum.tile([P, 512], F32, tag="o", bufs=2,
                             name=f"pso{p_}_{qc}_{i}")
                   for i in range(2)]
            nkt = 4 * (qc + 1)
            # AV runs two blocks behind exp: by the time an AV reaches the
            # head of the PE FIFO its exp semaphore is already satisfied, so
            # the in-order engine queue never stalls on the exp edge
            pending = []
            for kt in range(nkt):
                o = max(0, kt * P - qc * 512)
                kT = qkT[p_][1][kt // 4]
                kcol = (kt % 4) * P
                ss = psum.tile([P, 1024], F32, tag="s", bufs=2,
                               name=f"pss{p_}_{qc}_{kt}")
                for h in range(2):
                    lo, hi = h * 64, (h + 1) * 64
                    nc.tensor.matmul(
                        ss[:, 512 * h + o:512 * (h + 1)],
                        kT[lo:hi, kcol:kcol + P],
                        qT[lo:hi, o:512],
                    )
                pt = ptpool.tile([P, 1024], BF16, tag="pt",
                                 name=f"pt{p_}_{qc}_{kt}")
                ss3 = ss[:].rearrange("p (h w) -> p h w", h=2)
                pt3 = pt[:].rearrange("p (h w) -> p h w", h=2)
                nc.scalar.activation(
                    pt3[:, :, o:], ss3[:, :, o:],
                    mybir.ActivationFunctionType.Exp,
                )
                if kt >= 4 * qc:
                    # diagonal block: zero t_q < t_k (both heads)
                    mask_b = bass.AP(
                        tensor=mask[:].tensor, offset=mask[:].offset,
                        ap=[mask[:].ap[0], [0, 2], [1, P]])
                    nc.vector.tensor_mul(
                        pt3[:, :, o:o + P], pt3[:, :, o:o + P], mask_b)
                pending.append((pt, o, kt))
                if len(pending) > 2:
                    _emit_av(nc, vts, pso, p_, *pending.pop(0), nkt)
            for pend in pending:
                _emit_av(nc, vts, pso, p_, *pend, nkt)
            # normalize: pso rows 0:64 = row-sum, 64:128 = outT
            at = apool.tile([P, 512], BF16, tag=f"a{p_}_{qc}",
                            name=f"attnT{p_}_{qc}")
            attnT[p_][qc] = at
            rsb = rspool.tile([P, 1024], F32, tag="rs", name=f"rs{p_}_{qc}")
            # fast recip is lane-locked: compute at base 0 (frees the psum
            # fast), one combined DMA-shift to partitions 64-127
            nc.vector.reciprocal_approx_fast(rsb[0:64, 0:512], pso[0][0:64, :])
            nc.vector.reciprocal_approx_fast(rsb[0:64, 512:], pso[1][0:64, :])
            dma.dma_start(out=rsb[64:128, :], in_=rsb[0:64, :])
            for h in range(2):
                nc.vector.tensor_mul(
                    at[64 * h:64 * h + 64, :],
                    pso[h][64:128, :], rsb[64:128, 512 * h:512 * h + 512])

        def proj_chunk(tcn):
            for mt in range(NMT):
                psp = psum.tile([P, 512], F32, tag="mm", bufs=2,
                                name=f"psp{mt}_{tcn}")
                for jt in range(NPJ):
                    nc.tensor.matmul(
                        psp[:], wproj[jt][:, mt * P:(mt + 1) * P],
                        attnT[jt][tcn][:],
                        start=(jt == 0), stop=(jt == NPJ - 1),
                    )
                ot = opool.tile([P, 512], BF16, tag="ot", name=f"ot{mt}_{tcn}")
                nc.vector.tensor_scalar_add(out=ot[:], in0=psp[:], scalar1=0.0)
                dma.dma_start(
                    out=out_ext[mt * P:(mt + 1) * P,
                                tcn * 512:(tcn + 1) * 512],
                    in_=ot[:])

        # ---- main emission. The attention pipeline (scores/exp/AV/
        # normalize) is emitted under high_priority so the list scheduler
        # treats everything else (v remainder, qk of later pairs, proj) as
        # pure PE-slack filler -- the ACT exp chain is the serial resource
        # that must never wait.
        def attention_hp(p_, qc):
            with tc.high_priority(offset=1_000_000):
                attention(p_, qc)

        qkT[0] = [[None] * NTC, [None] * NTC]
        qk_group(0, 0)          # just tcn0: unblocks attention(0,0) asap
        for kt in range(4):
            v_tile(kt)
        attention_hp(0, 0)
        for tcn in range(1, NTC):
            qk_group(0, tcn)
        for qc in range(1, NTC):
            # v tiles for this chunk, emitted just ahead of it (deps only;
            # priority-wise they are filler)
            for kt in range(4 * qc, 4 * qc + 4):
                v_tile(kt)
            attention_hp(0, qc)
        for p_ in (1, 2):
            qk_tiles(p_)
            for qc in range(NTC):
                attention_hp(p_, qc)
        qk_tiles(3)
        # pair 3 runs q-chunks high-to-low; each chunk completes attnT[*][qc]
        # so its projection chunk is emitted (and runs) immediately after,
        # leaving only proj(qc=0) as the tail
        for qc in range(NTC - 1, -1, -1):
            attention_hp(3, qc)
            proj_chunk(qc)


def _emit_av(nc, vts, pso, p_, pt, o, kt, nkt):
    """attn@V for one (pair, kt) block: [ones|V_h].T @ P~ accumulated."""
    for h in range(2):
        head = 2 * p_ + h
        vaug = vts[kt][:, 128 * head:128 * head + 128]
        nc.tensor.matmul(
            pso[h][:, o:], vaug, pt[:, 512 * h + o:512 * (h + 1)],
            start=(kt == 0), stop=(kt == nkt - 1),
        )


def shard_inputs(x, W_qkv, b_qkv, W_proj, b_proj):
    """Build the 8 per-core input maps (host-side sharding)."""
    x = np.asarray(x, np.float32)
    W_qkv = np.asarray(W_qkv, np.float32)
    b_qkv = np.asarray(b_qkv, np.float32)
    W_proj = np.asarray(W_proj, np.float32)
    in_maps = []
    for c in range(NCORES):
        b, g = c // 2, c % 2
        s = slice(512 * g, 512 * g + 512)
        Wq = W_qkv[0 * C:1 * C][s] * 0.125
        Wk = W_qkv[1 * C:2 * C][s]
        Wv = W_qkv[2 * C:3 * C][s]
        bq = b_qkv[0 * C:1 * C][s] * 0.125
        bk = b_qkv[1 * C:2 * C][s]
        # pair-major interleave: [q0|k0|q1|k1|q2|k2|q3|k3|v]
        qk_rows = []
        bqk = []
        for p_ in range(4):
            qk_rows.append(Wq[128 * p_:128 * (p_ + 1)])
            qk_rows.append(Wk[128 * p_:128 * (p_ + 1)])
            bqk.append(bq[128 * p_:128 * (p_ + 1)])
            bqk.append(bk[128 * p_:128 * (p_ + 1)])
        wqkv = np.ascontiguousarray(
            np.concatenate(qk_rows + [Wv], 0).T).astype(ml_dtypes.bfloat16)
        bv = b_qkv[2 * C:3 * C][s]
        in_maps.append({
            "xT": np.ascontiguousarray(x[b].T).astype(ml_dtypes.bfloat16),
            "wqkv": wqkv,
            "bqkv": np.ascontiguousarray(np.concatenate(bqk + [bv])),
            "wproj": np.ascontiguousarray(W_proj[:, s].T).astype(ml_dtypes.bfloat16),
        })
    return in_maps


def run(in_maps, trace=False):
    global _compiled
    if _compiled is None:
        _compiled = build()
    return run_bass_kernel_spmd(
        _compiled, in_maps, core_ids=list(range(NCORES)), trace=trace)


def postprocess(res, b_proj):
    b_proj = np.asarray(b_proj, np.float32)
    out = np.empty((B, T, C), np.float32)
    for b in range(B):
        partial = (res.results[2 * b]["out"].astype(np.float32)
                   + res.results[2 * b + 1]["out"].astype(np.float32))
        out[b] = partial.T + b_proj
    return out


def kernel(x, W_qkv, b_qkv, W_proj, b_proj):
    in_maps = shard_inputs(x, W_qkv, b_qkv, W_proj, b_proj)
    res = run(in_maps)
    return postprocess(res, b_proj)


if __name__ == "__main__":
    rng = np.random.default_rng(0)
    xs = {
        "x": rng.standard_normal((B, T, C)).astype(np.float32),
        "W_qkv": (rng.standard_normal((3 * C, C)) / 32).astype(np.float32),
        "b_qkv": (rng.standard_normal(3 * C) * 0.02).astype(np.float32),
        "W_proj": (rng.standard_normal((C, C)) / 32).astype(np.float32),
        "b_proj": (rng.standard_normal(C) * 0.02).astype(np.float32),
    }
    out = kernel(**xs)
    print("out", out.shape, out.dtype, np.abs(out).mean())
